# revision 8
# baseline (speedup 1.0000x reference)
"""Causal multi-head attention block (B=2, T=2048, C=1024, H=16) on 8 TRN2 cores.

Sharding: tensor-parallel over heads x data-parallel over batch.
Core c handles batch b = c // 4 and head-group hg = c % 4 (4 heads = 256 of
the 1024 channel columns). Each core computes, for its batch and heads:
    QT/KT = (Wslice/8)^T X^T + b/8   (scores pre-scaled by 1/sqrt(D))
    V     = X Wv_slice + bv
    S^T   = K Q^T (causal, streamed in 128x512 chunks), P = exp(S^T)
    O^T   = [V|1]^T P  -> rows 0..63 unnormalized output, row 64 softmax sum
    partial = (O^T normalized)^T @ Wo_rows_slice        [2048, 1024]
Host sums the 4 partials per batch and adds bo.

MM_DT selects the PE datapath: float32r (full-rate fp32 path, ~1.6e-4 rel
per matmul) or float32 (exact, 4x slower rows).
"""

from contextlib import ExitStack

import numpy as np

import concourse.bacc as bacc
import concourse.mybir as mybir
import concourse.tile as tile
from concourse.bass_utils import run_bass_kernel_spmd

B, T, C, H, D = 2, 2048, 1024, 16, 64
N_CORES = 8
HG = 4                  # head-groups (tensor parallel)
HPC = H // HG           # heads per core = 4
HD = HPC * D            # channel slice per core = 256
P = 128                 # partitions
NT = T // 512           # 4 i-tiles of 512
NIC = T // P            # 16 i-chunks of 128
NKC = C // P            # 8 contraction chunks of 128
F32 = mybir.dt.float32
F32R = mybir.dt.float32r
AF = mybir.ActivationFunctionType

MM_DT = F32R            # matmul datapath dtype (F32R fast / F32 exact)

_CACHE: dict = {}


def _build_program():
    nc = bacc.Bacc("TRN2", debug=False)

    XT = nc.dram_tensor("XT", [C, T], MM_DT, kind="ExternalInput").ap()
    WQ = nc.dram_tensor("WQ", [C, HD], MM_DT, kind="ExternalInput").ap()
    WK = nc.dram_tensor("WK", [C, HD], MM_DT, kind="ExternalInput").ap()
    WV = nc.dram_tensor("WV", [C, HD], MM_DT, kind="ExternalInput").ap()
    BQ = nc.dram_tensor("BQ", [2, P, 1], F32, kind="ExternalInput").ap()
    BK = nc.dram_tensor("BK", [2, P, 1], F32, kind="ExternalInput").ap()
    BV = nc.dram_tensor("BV", [1, HD], MM_DT, kind="ExternalInput").ap()
    WO = nc.dram_tensor("WO", [HD, C], MM_DT, kind="ExternalInput").ap()
    OUT = nc.dram_tensor("OUT", [T, C], F32, kind="ExternalOutput").ap()

    # Causal chunk mask: for diagonal chunk k (k=0..3), valid iff f >= p + 128k,
    # realized as slices of Mbig[p, x] = (x >= p + 384).
    mb = (np.arange(896)[None, :] >= np.arange(P)[:, None] + 384).astype(np.float32)
    MBIG = nc.inline_tensor(mb, name="mbig").ap()
    ONES = nc.inline_tensor(np.ones((1, P), np.float32), name="ones_c").ap()
    VONES = nc.inline_tensor(np.ones((P, NIC * HPC), np.float32), name="vones_c").ap()

    with tile.TileContext(nc) as tc:
        _trace_kernel(tc, XT, WQ, WK, WV, BQ, BK, BV, WO, OUT, MBIG, ONES, VONES)
    nc.compile()
    return nc


def _trace_kernel(tc, XT, WQ, WK, WV, BQ, BK, BV, WO, OUT, MBIG, ONES, VONES):
    nc = tc.nc

    with ExitStack() as ctx:
        consts = ctx.enter_context(tc.tile_pool(name="consts", bufs=1))
        wpool = ctx.enter_context(tc.tile_pool(name="weights", bufs=1))
        xpool = ctx.enter_context(tc.tile_pool(name="xt", bufs=1))
        qkv = ctx.enter_context(tc.tile_pool(name="qkv", bufs=1))

        # ---- constants ----
        mbig_sb = consts.tile([P, 896], MM_DT, name="mbig_sb")
        nc.sync.dma_start(mbig_sb, MBIG.bitcast(MM_DT))
        ones_sb = consts.tile([1, P], MM_DT, name="ones_sb")
        nc.sync.dma_start(ones_sb, ONES.bitcast(MM_DT))
        bq_sb = consts.tile([P, 2], F32, name="bq_sb")
        bk_sb = consts.tile([P, 2], F32, name="bk_sb")
        for m in range(2):
            nc.sync.dma_start(bq_sb[:, m : m + 1], BQ[m])
            nc.sync.dma_start(bk_sb[:, m : m + 1], BK[m])
        bv_sb = consts.tile([1, HD], MM_DT, name="bv_sb")
        nc.sync.dma_start(bv_sb, BV)

        # ---- weights ----
        wq_sb = wpool.tile([P, NKC, HD], MM_DT, name="wq_sb")
        wk_sb = wpool.tile([P, NKC, HD], MM_DT, name="wk_sb")
        wv_sb = wpool.tile([P, NKC, HD], MM_DT, name="wv_sb")
        for kc in range(NKC):
            nc.sync.dma_start(wq_sb[:, kc, :], WQ[kc * P : (kc + 1) * P, :])
            nc.sync.dma_start(wk_sb[:, kc, :], WK[kc * P : (kc + 1) * P, :])
            nc.sync.dma_start(wv_sb[:, kc, :], WV[kc * P : (kc + 1) * P, :])
        wo_sb = wpool.tile([P, 2, C], MM_DT, name="wo_sb")
        for kc in range(2):
            nc.sync.dma_start(wo_sb[:, kc, :], WO[kc * P : (kc + 1) * P, :])

        # ---- X^T ----
        xts = [
            xpool.tile([P, T], MM_DT, name=f"xt{kc}", tag=f"xt{kc}")
            for kc in range(NKC)
        ]
        for kc in range(NKC):
            nc.sync.dma_start(xts[kc], XT[kc * P : (kc + 1) * P, :])

        # ---- persistent activations ----
        qt_sb = [qkv.tile([P, T], MM_DT, name=f"qt{m}", tag=f"qt{m}") for m in range(2)]
        kt_sb = [qkv.tile([P, T], MM_DT, name=f"kt{m}", tag=f"kt{m}") for m in range(2)]
        v_sb = qkv.tile([P, NIC, HPC, D + 1], MM_DT, name="v_sb")
        ot_sb = [qkv.tile([P, T], MM_DT, name=f"ot{m}", tag=f"ot{m}") for m in range(2)]
        nc.sync.dma_start(v_sb[:, :, :, D : D + 1], VONES.bitcast(MM_DT))

        # ---- stage A: projections ----
        with tc.tile_pool(name="psA", bufs=2, space="PSUM") as psA:
            for t in range(NT):
                sl = slice(512 * t, 512 * (t + 1))
                for m in range(2):
                    msl = slice(P * m, P * (m + 1))
                    pq = psA.tile([P, 512], F32, tag="pq")
                    for kc in range(NKC):
                        nc.tensor.matmul(
                            pq,
                            lhsT=wq_sb[:, kc, msl],
                            rhs=xts[kc][:, sl],
                            start=(kc == 0),
                            stop=(kc == NKC - 1),
                        )
                    nc.scalar.activation(
                        qt_sb[m][:, sl], pq, AF.Identity, bias=bq_sb[:, m : m + 1]
                    )
                    pk = psA.tile([P, 512], F32, tag="pk")
                    for kc in range(NKC):
                        nc.tensor.matmul(
                            pk,
                            lhsT=wk_sb[:, kc, msl],
                            rhs=xts[kc][:, sl],
                            start=(kc == 0),
                            stop=(kc == NKC - 1),
                        )
                    nc.scalar.activation(
                        kt_sb[m][:, sl], pk, AF.Identity, bias=bk_sb[:, m : m + 1]
                    )
                for ic in range(4 * t, 4 * (t + 1)):
                    isl = slice(P * ic, P * (ic + 1))
                    pv = psA.tile([P, HD], F32, tag="pv")
                    for kc in range(NKC):
                        nc.tensor.matmul(
                            pv,
                            lhsT=xts[kc][:, isl],
                            rhs=wv_sb[:, kc, :],
                            start=(kc == 0),
                            stop=False,
                        )
                    nc.tensor.matmul(
                        pv, lhsT=ones_sb, rhs=bv_sb, start=False, stop=True
                    )
                    nc.scalar.copy(
                        v_sb[:, ic, :, 0:D], pv.rearrange("p (h d) -> p h d", d=D)
                    )

        # ---- stages B+C: attention + output projection ----
        drsc = ctx.enter_context(tc.tile_pool(name="drsc", bufs=2, space="DRAM"))
        psB = ctx.enter_context(tc.tile_pool(name="psB", bufs=2, space="PSUM"))
        psC = ctx.enter_context(tc.tile_pool(name="psC", bufs=2, space="PSUM"))
        spool = ctx.enter_context(tc.tile_pool(name="spool", bufs=3))
        npool = ctx.enter_context(tc.tile_pool(name="npool", bufs=2))
        opool = ctx.enter_context(tc.tile_pool(name="opool", bufs=2))

        for t in range(NT):
            sl = slice(512 * t, 512 * (t + 1))
            njc = 4 * (t + 1)
            for l in range(HPC):
                mc, ro = l // 2, 64 * (l % 2)
                qrow = slice(ro, ro + 64)
                pot = psB.tile([D + 1, 512], F32, tag="pot")
                for jcp in range(njc // 2):
                    ps = psB.tile([P, 1024], F32, tag="ps")
                    for half in (0, 1):
                        jc = 2 * jcp + half
                        nc.tensor.matmul(
                            ps[:, 512 * half : 512 * (half + 1)],
                            lhsT=kt_sb[mc][qrow, P * jc : P * (jc + 1)],
                            rhs=qt_sb[mc][qrow, sl],
                            start=True,
                            stop=True,
                        )
                    ex = spool.tile([P, 1024], MM_DT, tag="ex")
                    nc.scalar.activation(ex, ps, AF.Exp)
                    for half in (0, 1):
                        jc = 2 * jcp + half
                        exh = ex[:, 512 * half : 512 * (half + 1)]
                        k = jc - 4 * t
                        if k >= 0:  # diagonal chunk: zero out j > i
                            nc.vector.tensor_mul(
                                exh, exh, mbig_sb[:, 384 - 128 * k : 896 - 128 * k]
                            )
                        nc.tensor.matmul(
                            pot,
                            lhsT=v_sb[:, jc, l, 0 : D + 1],
                            rhs=exh,
                            start=(jc == 0),
                            stop=(jc == njc - 1),
                        )
                # normalize by softmax sum (row D of pot)
                rc = npool.tile([1, 512], F32, tag="rc")
                nc.vector.reciprocal(rc, pot[D : D + 1, :])
                rd = drsc.tile([1, 512], F32, tag="rd")
                nc.sync.dma_start(rd, rc)
                bc = npool.tile([64, 512], F32, tag="bc")
                nc.sync.dma_start(bc, rd.to_broadcast((64, 512)))
                nc.vector.tensor_mul(ot_sb[mc][qrow, sl], pot[0:D, :], bc)

            # output projection for the 4 i-chunks of this i-tile
            for ic in range(4 * t, 4 * (t + 1)):
                isl = slice(P * ic, P * (ic + 1))
                ob = opool.tile([P, C], F32, tag="ob")
                for n in (0, 1):
                    po = psC.tile([P, 512], F32, tag="po")
                    for kc in range(2):
                        nc.tensor.matmul(
                            po,
                            lhsT=ot_sb[kc][:, isl],
                            rhs=wo_sb[:, kc, 512 * n : 512 * (n + 1)],
                            start=(kc == 0),
                            stop=(kc == 1),
                        )
                    if n == 0:
                        nc.scalar.copy(ob[:, 0:512], po)
                    else:
                        nc.vector.tensor_copy(ob[:, 512:1024], po)
                nc.sync.dma_start(OUT[isl, :], ob)


def _get_program():
    if "nc" not in _CACHE:
        _CACHE["nc"] = _build_program()
    return _CACHE["nc"]


def _shard_inputs(X, Wq, bq, Wk, bk, Wv, bv, Wo, bo):
    in_maps = []
    for c in range(N_CORES):
        b, hg = divmod(c, HG)
        cols = slice(HD * hg, HD * (hg + 1))
        in_maps.append(
            {
                "XT": np.ascontiguousarray(X[b].T),
                "WQ": np.ascontiguousarray(Wq[:, cols]) * 0.125,
                "WK": np.ascontiguousarray(Wk[:, cols]),
                "WV": np.ascontiguousarray(Wv[:, cols]),
                "BQ": (bq[cols] * 0.125).reshape(2, P, 1).astype(np.float32),
                "BK": bk[cols].reshape(2, P, 1).astype(np.float32),
                "BV": bv[cols].reshape(1, HD).astype(np.float32),
                "WO": np.ascontiguousarray(Wo[cols, :]),
            }
        )
    return in_maps


def kernel(X, Wq, bq, Wk, bk, Wv, bv, Wo, bo):
    X = np.asarray(X, dtype=np.float32)
    Wq, bq = np.asarray(Wq, np.float32), np.asarray(bq, np.float32)
    Wk, bk = np.asarray(Wk, np.float32), np.asarray(bk, np.float32)
    Wv, bv = np.asarray(Wv, np.float32), np.asarray(bv, np.float32)
    Wo, bo = np.asarray(Wo, np.float32), np.asarray(bo, np.float32)

    nc = _get_program()
    in_maps = _shard_inputs(X, Wq, bq, Wk, bk, Wv, bv, Wo, bo)
    res = run_bass_kernel_spmd(nc, in_maps, core_ids=list(range(N_CORES))).results

    out = np.empty((B, T, C), dtype=np.float32)
    for b in range(B):
        acc = np.zeros((T, C), dtype=np.float64)
        for hg in range(HG):
            acc += res[HG * b + hg]["OUT"]
        out[b] = (acc + bo.astype(np.float64)).astype(np.float32)
    return out
